# revision 57
# baseline (speedup 1.0000x reference)
"""AdaMoLE (LoRA-MoE routing) Trainium2 kernel, data-parallel over tokens on 8 cores.

Math (per token n):
    logits = x @ Wr.T + br                 [E]
    gate   = softmax(logits)
    thr    = sigmoid(x @ Wt.T + bt) / E    [1]
    w      = relu(gate - thr); w /= max(sum(w), eps)
    h      = x @ A_all                     [E*R]   (A_all = concat_e A_e)
    out    = (h * rep(w) * SCALING) @ B_all        (B_all = concat_e B_e)

Key restructurings vs the straightforward version:
  * Scale cancellation: w = relu(gate - thr)/sum(...) is invariant to the
    softmax denominator S, so we use w' = relu(eexp - thr*S) and divide by
    sum(w') instead -- no gate normalization broadcast needed.
  * The final 1/sum(w') is applied AFTER the second matmul, where tokens sit
    on partitions, as a per-partition scalar in the PSUM->SBUF copy.
  * All PE operands are bf16 (f32 matmuls run 4x slower); sigmoid goes
    through Exp so the scalar engine keeps one activation table loaded.
  * Software-pipelined emission: block k+1's router matmuls are emitted in
    two halves inside block k (after the threshold broadcast, and between
    the weight-broadcast matmul and mm2) so the PE never idles on the
    cross-engine ACT/DVE latency of the routing chain.
  * 0.5 MB sub-loads / 0.25 MB sub-stores: Tile's range-based dependency
    tracking lets the PE consume x chunks while the rest of the block
    streams in, and stores trickle out right behind the PSUM copies.

Each core takes 2048 tokens in 512-token blocks (the tail split finer to
shorten the pipeline drain). x arrives host-side pre-cast to bf16 in a
transposed chunk-major layout (halves HBM read); the output is written
bf16 in a block-major layout and unshuffled + upcast on the host. X loads
and output stores ride the sync HWDGE ring; weights ride the scalar HWDGE
ring (no SWDGE anywhere: cheapens the engine drain). Triple-buffered
output staging lets stores lag a full block behind the PSUM copies.
"""

import sys

sys.path.insert(0, "/opt/trn_rl_repo")

import numpy as np
import ml_dtypes

import concourse.bacc as bacc
import concourse.mybir as mybir
import concourse.tile as tile
from concourse.bass_utils import run_bass_kernel_spmd
from contextlib import ExitStack

F32 = mybir.dt.float32
BF16 = mybir.dt.bfloat16
AF = mybir.ActivationFunctionType
ALU = mybir.AluOpType

B, S, D, DOUT = 4, 4096, 4096, 4096
R, E = 16, 8
SCALING = 8.0 / R  # lora_alpha / r
NCORES = 8
N = B * S
NTOK = N // NCORES        # 2048 tokens per core
NDC = D // 128            # 32 contraction chunks
ER = E * R                # 128
BLOCKS = [512, 512, 512, 384, 128]   # token block sizes (sum = NTOK)
NBLK = len(BLOCKS)
T0S = [sum(BLOCKS[:i]) for i in range(NBLK)]          # token offsets
OPBS = [(bs // 128) * DOUT for bs in BLOCKS]          # OUT cols per block
OOFF = [sum(OPBS[:i]) for i in range(NBLK)]           # OUT col offsets
LSUBW = 2048              # X sub-load width (cols; 0.5 MB)
SSUBW = 1024              # OUT sub-store width (cols; 0.25 MB)

_CACHE = {}


def _build(reps=1, loop=False):
    nc = bacc.Bacc("TRN2", debug=False, num_devices=NCORES)

    X = nc.declare_dram_parameter("X", [128, NDC * NTOK], BF16, isOutput=False)
    Aw = nc.declare_dram_parameter("Aw", [128, NDC * ER], BF16, isOutput=False)
    Wc = nc.declare_dram_parameter("Wc", [128, NDC * 9], BF16, isOutput=False)
    Bl = nc.declare_dram_parameter("Bl", [ER, DOUT], BF16, isOutput=False)
    # SMB packs the small bf16 consts: [0:8,0:128]=REP (w->er replicate, pre-
    # scaled by SCALING), [0:8,128:129]=ones[8,1], [0,129:137]=ones[1,8],
    # [0:9,137:138]=row-8 selector
    SMB = nc.declare_dram_parameter("SMB", [9, 138], BF16, isOutput=False)
    # CB9 packs the Exp prologue: [:,0]=bias (br; -bt), [:,1]=scale (1x8; -1)
    CB9 = nc.declare_dram_parameter("CB9", [9, 2], F32, isOutput=False)
    OUT = nc.declare_dram_parameter("out", [128, (NTOK // 128) * DOUT], BF16,
                                    isOutput=True)

    with tile.TileContext(nc) as tc, ExitStack() as ctx:
        wpool = ctx.enter_context(tc.tile_pool(name="w", bufs=1))
        xpool = ctx.enter_context(tc.tile_pool(name="x", bufs=2))
        opool = ctx.enter_context(tc.tile_pool(name="o", bufs=3))
        spool = ctx.enter_context(tc.tile_pool(name="s", bufs=2))
        hwpool = ctx.enter_context(tc.tile_pool(name="hw", bufs=2))
        ph = ctx.enter_context(tc.tile_pool(name="ph", bufs=2, space="PSUM"))
        pr = ctx.enter_context(tc.tile_pool(name="pr", bufs=1, space="PSUM"))
        pm = ctx.enter_context(tc.tile_pool(name="pm", bufs=2, space="PSUM"))
        po = ctx.enter_context(tc.tile_pool(name="po", bufs=3, space="PSUM"))

        # weights on the scalar HWDGE ring: X loads start immediately on the
        # sync ring without queueing behind 2 MB of weights, and keeping
        # SWDGE entirely out of the program cheapens the end-of-program
        # (and For_i back-edge) engine drain
        Wc_sb = wpool.tile([128, NDC * 9], BF16, tag="Wc")
        nc.scalar.dma_start(out=Wc_sb[:], in_=Wc[:])
        A_sb = wpool.tile([128, NDC * ER], BF16, tag="A")
        nc.scalar.dma_start(out=A_sb[:], in_=Aw[:])
        SMB_sb = wpool.tile([9, 138], BF16, tag="SMB")
        nc.scalar.dma_start(out=SMB_sb[:], in_=SMB[:])
        CB9_sb = wpool.tile([9, 2], F32, tag="CB9")
        nc.scalar.dma_start(out=CB9_sb[:], in_=CB9[:])
        B_sb = wpool.tile([ER, DOUT], BF16, tag="B")
        nc.scalar.dma_start(out=B_sb[:], in_=Bl[:])

        REPb = SMB_sb[0:8, 0:128]
        ONESb = SMB_sb[0:8, 128:129]
        BC1b = SMB_sb[0:1, 129:137]
        SEL9b = SMB_sb[0:9, 137:138]
        BIAS9 = CB9_sb[:, 0:1]
        SCL9 = CB9_sb[:, 1:2]

        xb_t = [None] * NBLK
        rps_t = [None] * NBLK
        e9_t = [None] * NBLK
        s1_t = [None] * NBLK
        en0_t = [None] * NBLK

        def emit_load(k):
            bs = BLOCKS[k]
            cpb = NDC * bs
            x0 = NDC * T0S[k]
            xb = xpool.tile([128, cpb], BF16, tag="xb")
            xb_t[k] = xb
            for c0 in range(0, cpb, LSUBW):
                c1 = min(c0 + LSUBW, cpb)
                nc.sync.dma_start(out=xb[:, c0:c1], in_=X[:, x0 + c0 : x0 + c1])

        def emit_r(k, dc0=0, dc1=NDC):
            bs = BLOCKS[k]
            if dc0 == 0:
                r_ps = pr.tile([9, bs], F32, tag="r")
                rps_t[k] = r_ps
            r_ps = rps_t[k]
            xb = xb_t[k]
            for dc in range(dc0, dc1):
                nc.tensor.matmul(
                    r_ps[:],
                    Wc_sb[:, dc * 9 : (dc + 1) * 9],
                    xb[:, dc * bs : (dc + 1) * bs],
                    start=(dc == 0),
                    stop=(dc == NDC - 1),
                )

        def emit_e9(k):
            # e9[0:8] = exp(logits + br); e9[8] = exp(-(rt + bt))
            bs = BLOCKS[k]
            e9n = spool.tile([9, bs], BF16, tag="e9")
            nc.scalar.activation(e9n[:], rps_t[k][:], AF.Exp, bias=BIAS9, scale=SCL9)
            e9_t[k] = e9n

        def emit_block(k):
            bs = BLOCKS[k]
            ng = bs // 128
            opb = OPBS[k]
            xb = xb_t[k]
            r_ps = rps_t[k]

            # S1 / en0 first: they only need e9 (already done), and issuing
            # them before h lets the DVE chain overlap the whole h loop
            e9p = e9_t[k]
            S1p = pm.tile([1, bs], F32, tag="pm")
            nc.tensor.matmul(S1p[:], ONESb, e9p[0:8, :], start=True, stop=True)
            s1_t[k] = S1p
            en0p = pm.tile([1, bs], F32, tag="pm")
            nc.tensor.matmul(en0p[:], SEL9b, e9p[:], start=True, stop=True)
            en0_t[k] = en0p

            # h[er, t] accumulated over the 32 d-chunks
            h_ps = ph.tile([ER, bs], F32, tag="h")
            for dc in range(NDC):
                nc.tensor.matmul(
                    h_ps[:],
                    A_sb[:, dc * ER : (dc + 1) * ER],
                    xb[:, dc * bs : (dc + 1) * bs],
                    start=(dc == 0),
                    stop=(dc == NDC - 1),
                )

            # ---- routing, front half ----
            # e9 was computed at the end of the previous block (ACT overlapped
            # with its mm2); S1/en0 were emitted before the h loop, so the DVE
            # den->rec->thrS chain below runs fully under h's shadow
            e9 = e9_t[k]
            S1 = s1_t[k]
            en0 = en0_t[k]
            # den = E * (1 + exp(-(rt+bt)))  ==  E / sigmoid(rt+bt)
            den = spool.tile([1, bs], F32, tag="den")
            nc.vector.tensor_scalar(den[:], en0[:], 1.0, float(E), ALU.add, ALU.mult)
            rec = spool.tile([1, bs], F32, tag="rec")
            nc.vector.reciprocal(rec[:], den[:])
            # thrS = sigmoid(rt+bt)/E * S  (threshold in the unnormalized space)
            thrS = spool.tile([1, bs], BF16, tag="thrS")
            nc.vector.tensor_mul(thrS[:], S1[:], rec[:])
            TH8 = pm.tile([8, bs], F32, tag="pm")
            nc.tensor.matmul(TH8[:], BC1b, thrS[:], start=True, stop=True)

            # first half of block k+1's router matmuls slots in here: the PE
            # chews on them while the DVE finishes this block's routing chain
            if k + 2 < NBLK:
                emit_load(k + 2)
            if k + 1 < NBLK:
                emit_r(k + 1, 0, NDC // 2)

            # ---- routing, back half ----
            wsub = spool.tile([8, bs], F32, tag="wsub")
            nc.vector.scalar_tensor_tensor(
                wsub[:], TH8[:], -1.0, e9[0:8, :], ALU.mult, ALU.add
            )
            wrelu = spool.tile([8, bs], BF16, tag="wrelu")
            nc.vector.tensor_scalar_max(wrelu[:], wsub[:], 0.0)
            # per-token sum of selected weights, tokens on partitions
            S2T = pm.tile([128, ng], F32, tag="pm")
            for g in range(ng):
                nc.tensor.matmul(
                    S2T[:, g : g + 1],
                    wrelu[:, g * 128 : (g + 1) * 128],
                    ONESb,
                    start=True,
                    stop=True,
                )
            clmp = spool.tile([128, ng], F32, tag="clmp")
            nc.vector.tensor_scalar_max(clmp[:], S2T[:], 1e-30)
            srecT = spool.tile([128, ng], F32, tag="srecT")
            nc.vector.reciprocal(srecT[:], clmp[:])
            WREPp = pm.tile([ER, bs], F32, tag="pm")
            nc.tensor.matmul(WREPp[:], REPb, wrelu[:], start=True, stop=True)
            WREP = spool.tile([ER, bs], BF16, tag="WREP")
            nc.scalar.activation(WREP[:], WREPp[:], AF.Copy)
            hw = hwpool.tile([ER, bs], BF16, tag="hw")
            nc.vector.tensor_mul(hw[:], WREP[:], h_ps[:])
            # second half of block k+1's router matmuls HERE: the PE chews on
            # them while ACT copies WREP and the DVE computes hw, so the first
            # mm2 matmul never waits on that cross-engine latency
            if k + 1 < NBLK:
                emit_r(k + 1, NDC // 2, NDC)
                emit_e9(k + 1)

            # ---- second matmul + scaled output copy (bf16, block-major) ----
            o_sb = opool.tile([128, opb], BF16, tag="osb")
            for t4 in range(ng):
                sc = srecT[:, t4 : t4 + 1]
                for nb in range(DOUT // 512):
                    o_ps = po.tile([128, 512], F32, tag="o")
                    nc.tensor.matmul(
                        o_ps[:],
                        hw[:, t4 * 128 : (t4 + 1) * 128],
                        B_sb[:, nb * 512 : (nb + 1) * 512],
                        start=True,
                        stop=True,
                    )
                    dst = o_sb[:, t4 * DOUT + nb * 512 : t4 * DOUT + (nb + 1) * 512]
                    if nb % 2 == 0:
                        nc.scalar.activation(dst, o_ps[:], AF.Copy, scale=sc)
                    else:
                        nc.vector.tensor_scalar_mul(dst, o_ps[:], sc)
            for c0 in range(0, opb, SSUBW):
                c1 = min(c0 + SSUBW, opb)
                nc.sync.dma_start(
                    out=OUT[:, OOFF[k] + c0 : OOFF[k] + c1], in_=o_sb[:, c0:c1]
                )

        def emit_all():
            emit_load(0)
            if NBLK > 1:
                emit_load(1)
            emit_r(0)
            emit_e9(0)
            for k in range(NBLK):
                emit_block(k)

        if loop:
            # weights stay loaded across iterations; loop body is HWDGE-only
            # (SWDGE DMAs in flight at the For_i back-edge can fault the device)
            body_reps, trip = loop if isinstance(loop, tuple) else (1, reps)
            with tc.For_i(0, trip, 1):
                for r in range(body_reps):
                    emit_all()
        else:
            for r in range(reps):
                emit_all()

    nc.compile()
    return nc


def _prep_consts(Wr, br, Wt, bt, A, Bw):
    bf = ml_dtypes.bfloat16
    A_all = np.ascontiguousarray(
        np.asarray(A, np.float32).transpose(1, 0, 2).reshape(D, ER)
    )  # [d, er]
    A_host = np.ascontiguousarray(
        A_all.reshape(NDC, 128, ER).transpose(1, 0, 2).reshape(128, NDC * ER)
    ).astype(bf)
    Wcat = np.concatenate(
        [np.asarray(Wr, np.float32).T, np.asarray(Wt, np.float32).T], axis=1
    )  # [d, 9]
    Wc_host = np.ascontiguousarray(
        Wcat.reshape(NDC, 128, 9).transpose(1, 0, 2).reshape(128, NDC * 9)
    ).astype(bf)
    B_host = np.ascontiguousarray(np.asarray(Bw, np.float32).reshape(ER, DOUT)).astype(bf)
    SMBh = np.zeros((9, 138), np.float32)
    for e in range(E):
        SMBh[e, e * R : (e + 1) * R] = SCALING
    SMBh[0:8, 128] = 1.0
    SMBh[0, 129:137] = 1.0
    SMBh[8, 137] = 1.0
    CB9h = np.zeros((9, 2), np.float32)
    CB9h[0:8, 0] = np.asarray(br, np.float32).reshape(E)
    CB9h[8, 0] = -np.float32(np.asarray(bt).reshape(()))
    CB9h[0:8, 1] = 1.0
    CB9h[8, 1] = -1.0
    return {
        "Aw": A_host,
        "Wc": Wc_host,
        "Bl": B_host,
        "SMB": SMBh.astype(bf),
        "CB9": CB9h,
    }


def _prep_x(xs):
    """Per-core shard [NTOK, D] (bf16) -> [128, NDC*NTOK] with per-block
    [p, dc, t] layout so every DMA slice is contiguous."""
    parts = []
    for k in range(NBLK):
        t0, bs = T0S[k], BLOCKS[k]
        blkarr = (
            xs[t0 : t0 + bs]
            .reshape(bs, NDC, 128)
            .transpose(2, 1, 0)
            .reshape(128, NDC * bs)
        )
        parts.append(blkarr)
    return np.ascontiguousarray(np.concatenate(parts, axis=1))


def _unshard_out(oarr):
    """Device layout [128, sum(OPBS)] (bf16) -> [NTOK, DOUT] f32."""
    o = np.asarray(oarr)
    rows = []
    for k in range(NBLK):
        ng = BLOCKS[k] // 128
        blk = o[:, OOFF[k] : OOFF[k] + OPBS[k]].reshape(128, ng, DOUT)
        rows.append(blk.transpose(1, 0, 2).reshape(BLOCKS[k], DOUT))
    return np.concatenate(rows, axis=0).astype(np.float32)


def kernel(x, Wr, br, Wt, bt, A, Bw, _trace=False, _trace_kwargs=None):
    if "nc" not in _CACHE:
        _CACHE["nc"] = _build()
    nc = _CACHE["nc"]

    consts = _prep_consts(Wr, br, Wt, bt, A, Bw)
    xf = np.asarray(x, np.float32).reshape(N, D).astype(ml_dtypes.bfloat16)
    in_maps = []
    for c in range(NCORES):
        Xh = _prep_x(xf[c * NTOK : (c + 1) * NTOK])
        in_maps.append({"X": Xh, **consts})

    res = run_bass_kernel_spmd(
        nc,
        in_maps,
        core_ids=list(range(NCORES)),
        trace=_trace,
        **(_trace_kwargs or {}),
    )
    out = np.concatenate(
        [_unshard_out(res.results[c]["out"]) for c in range(NCORES)], axis=0
    )
    if _trace:
        _CACHE["last_res"] = res
    return out.reshape(B, S, DOUT)
